# revision 1
# baseline (speedup 1.0000x reference)
"""CapsuleLayer dynamic-routing kernel for 8 Trainium2 NeuronCores.

Problem (hardcoded shapes):
  x: [B=64, R=2048, I=16] f32, W: [R=2048, C=16, O=32, I=16] f32
  u_hat[b,r,c,o] = sum_i W[r,c,o,i] * x[b,r,i]
  3 dynamic-routing iterations (softmax over c, squash over o) -> v [B, R, O]

Strategy:
  - Shard R across 8 cores (256 r's each). No collectives needed.
  - Host-side layout prep (not counted in HW time):
      * xblk[rp, 32, 128]: block-diag stationary for a pair of r's
        (K=(r_hat,i)=32, M=(r_hat,b)=128)
      * wm[rp, 32, 544]: moving operand: W[r,i,(c,o)] for the pair, plus 32
        extra columns holding mean_c W (folds iteration-0's uniform-softmax
        contraction into the same matmul).
  - Device: per r-pair chunk, PE computes u_hat [128=(r_hat,b), 512=(c,o)]
    and s0 [128, 32] in PSUM; routing runs on DVE/ACT/GPSIMD in fp32
    (bf16/tf32 break the routing: softmax logits ~ +-40 amplify errors).
"""

import numpy as np
import sys

sys.path.insert(0, "/opt/trn_rl_repo")

B, R, C, O, I = 64, 2048, 16, 32, 16
N_CORES = 8
R_SHARD = R // N_CORES          # 256
NPAIR = R_SHARD // 2            # 128 chunks per core
N_ITER = 3

_cache = {}


def _build_program(npair=NPAIR, reps=1, variant="dvemul_hwdma_g8_gps2"):
    """Build the Bass program once; returns nc. reps>1 repeats the whole
    computation (idempotent) for wall-clock-delta timing.
    variant: 'full' | 'nort' (no routing) | 'dvemul' (both big muls on DVE)
             | 'gpsmul' (both big muls on GPSIMD) | 'noalpha' (skip alpha
             chains, v_scale=const) | 'hwdma' (sync-engine DMA)."""
    from contextlib import ExitStack

    import concourse.bacc as bacc
    import concourse.tile as tile
    from concourse import mybir

    # The act-table-load pass assigns each activation the FIRST table set
    # containing its func: Copy/Exp/Square -> set 0, Ln -> set 5, causing a
    # ~2.7us table reload on nearly every activation. All four funcs coexist
    # in set "natural_log_exp_and_others"; blank out earlier sets (indices
    # must be preserved - they index the real act_info.json) so everything
    # lands on that one set => a single table load for the whole kernel.
    if not getattr(bacc, "_act_tables_patched", False):
        _orig_get_tables = bacc.get_activation_tables

        def _patched(arch):
            tabs = dict(_orig_get_tables(arch))
            target = "natural_log_exp_and_others"
            assert target in tabs
            return {
                name: (funcs if name == target else set())
                for name, funcs in tabs.items()
            }

        bacc.get_activation_tables = _patched
        bacc._act_tables_patched = True

    f32 = mybir.dt.float32
    AX = mybir.AxisListType
    ALU = mybir.AluOpType
    ACTF = mybir.ActivationFunctionType

    nc = bacc.Bacc("TRN2", target_bir_lowering=False, debug=False)

    # xw[rp, 32, 672]: [:, :, :128] = block-diag x stationary, [:, :, 128:672]
    # = W moving operand (512 u_hat cols + 32 mean_c-W cols). One DMA per
    # chunk => a single wait on each matmul (walrus sync-slot limit).
    xw = nc.dram_tensor("xw", [npair, 32, 672], f32, kind="ExternalInput")
    vout = nc.dram_tensor("vout", [B, 2 * npair, O], f32, kind="ExternalOutput")

    xw_ap = xw.ap()
    # view: [rp, r_hat, b, o] so a [128=(r_hat,b), 32] tile DMAs straight out
    vout_view = vout.ap().rearrange("b (rp two) o -> rp two b o", two=2)

    with tile.TileContext(nc) as tc, ExitStack() as ctx:
        xp = ctx.enter_context(tc.tile_pool(name="xp", bufs=16))
        nA = 6 if "psa6" in variant else 4
        psA = ctx.enter_context(tc.tile_pool(name="psA", bufs=nA, space="PSUM"))
        psB = ctx.enter_context(tc.tile_pool(name="psB", bufs=8 - nA, space="PSUM"))
        up = ctx.enter_context(tc.tile_pool(name="up", bufs=14))
        tp = ctx.enter_context(tc.tile_pool(name="tp", bufs=12))
        sp = ctx.enter_context(tc.tile_pool(name="sp", bufs=14))
        sm = ctx.enter_context(tc.tile_pool(name="sm", bufs=4))

        dma_eng = nc.sync if "hwdma" in variant else nc.gpsimd
        G = 16 if "g16" in variant else (8 if "g8" in variant else 4)
        while (npair * reps) % G:
            G //= 2

        def alpha_batch(squF, ZF, tagp):
            """Batched over a group: alpha*zi [128,G] from ||s_un||^2 and Z.
            alpha = sqrt(sig2)/(1+sig2), sig2 = squF*zi^2 (zi=1 if ZF None).
            sqrt via exp(0.5*ln(x)): keeps every ACT func in ONE table set
            (natural_log_exp_and_others) — a Sqrt op would force a ~2.7us
            ACT table reload on every Exp<->Sqrt alternation."""
            if ZF is not None:
                zi = sm.tile([128, G], f32, tag=tagp + "zi")
                nc.vector.reciprocal(zi, ZF)
                zi2 = sm.tile([128, G], f32, tag=tagp + "zi2")
                nc.vector.tensor_mul(zi2, zi, zi)
                sig2 = sm.tile([128, G], f32, tag=tagp + "sig2")
                nc.vector.tensor_mul(sig2, squF, zi2)
            else:
                sig2 = squF
            a1 = sm.tile([128, G], f32, tag=tagp + "a1")
            nc.vector.tensor_scalar_add(a1, sig2, 1.0)
            ra = sm.tile([128, G], f32, tag=tagp + "ra")
            nc.vector.reciprocal(ra, a1)
            lt = sm.tile([128, G], f32, tag=tagp + "lt")
            nc.scalar.activation(lt, sig2, ACTF.Ln)
            rt = sm.tile([128, G], f32, tag=tagp + "rt")
            nc.scalar.activation(rt, lt, ACTF.Exp, scale=0.5)
            al = sm.tile([128, G], f32, tag=tagp + "al")
            nc.vector.tensor_mul(al, rt, ra)
            if ZF is not None:
                az = sm.tile([128, G], f32, tag=tagp + "az")
                nc.vector.tensor_mul(az, al, zi)
                return az
            return al

        rps = [i for _ in range(reps) for i in range(npair)]
        assert len(rps) % G == 0
        for g0 in range(0, len(rps), G):
            grp = rps[g0:g0 + G]

            # P0/P1: loads + matmuls
            us, ss = [], []
            for rp in grp:
                xwt = xp.tile([32, 672], f32)
                dma_eng.dma_start(out=xwt, in_=xw_ap[rp])
                u_ps = psA.tile([128, 512], f32)
                nc.tensor.matmul(u_ps, lhsT=xwt[:, :128], rhs=xwt[:, 128:640],
                                 start=True, stop=True)
                s0_ps = psB.tile([128, O], f32)
                nc.tensor.matmul(s0_ps, lhsT=xwt[:, :128], rhs=xwt[:, 640:],
                                 start=True, stop=True)
                us.append((u_ps, s0_ps))

            # P2: evacuate + ||s0||^2 into group state
            u_sb, s_cur = [], []
            squ0 = sm.tile([128, G], f32, tag="squ0")
            for j, (u_ps, s0_ps) in enumerate(us):
                u = up.tile([128, 512], f32)
                nc.scalar.copy(u, u_ps)      # ACT evacuates PSUM
                u_sb.append(u.rearrange("p (c o) -> p c o", o=O))
                junk = sp.tile([128, O], f32, tag="junk")
                if "s0psum" in variant:
                    # read s0 straight from PSUM (ACT PSUM-src is cheaper
                    # than SBUF-src); it-1's broadcast mul also reads PSUM
                    s_cur.append(s0_ps)
                    nc.scalar.activation(junk, s0_ps, ACTF.Square,
                                         accum_out=squ0[:, j:j + 1])
                else:
                    s = sp.tile([128, O], f32, tag="s0")
                    nc.scalar.copy(s, s0_ps)
                    s_cur.append(s)
                    nc.scalar.activation(junk, s, ACTF.Square,
                                         accum_out=squ0[:, j:j + 1])
            # P3: batched alpha0 (Z=1: c uniform via mean_c-W matmul columns)
            vsc = alpha_batch(squ0, None, "a0")

            b_cur = [None] * G
            for it in (1, 2):
                # P4/P6: per-chunk agreement + softmax + s_unnorm
                squF = sm.tile([128, G], f32, tag=f"squ{it}")
                ZF = sm.tile([128, G], f32, tag=f"Z{it}")
                mF = sm.tile([128, G], f32, tag=f"m{it}")
                nmF = sm.tile([128, G], f32, tag=f"nm{it}")
                s_next = []
                for j in range(G):
                    u3 = u_sb[j]
                    t1 = tp.tile([128, 16, O], f32, tag="t1")
                    s_b = s_cur[j].unsqueeze(1).broadcast_to((128, 16, O))
                    if ("gps2" in variant) or ("gps1" in variant and it == 2):
                        nc.gpsimd.tensor_tensor(t1, u3, s_b, op=ALU.mult)
                    else:
                        nc.vector.tensor_mul(t1, u3, s_b)
                    bd = sp.tile([128, 16], f32, tag="bd")
                    nc.vector.reduce_sum(bd, t1, axis=AX.X)
                    b_new = sp.tile([128, 16], f32, tag="bnew")
                    if b_cur[j] is None:
                        nc.vector.tensor_scalar_mul(b_new, bd, vsc[:, j:j + 1])
                    else:
                        nc.vector.scalar_tensor_tensor(
                            out=b_new, in0=bd, scalar=vsc[:, j:j + 1],
                            in1=b_cur[j], op0=ALU.mult, op1=ALU.add)
                    b_cur[j] = b_new
                    # negate=True: out = -max, directly usable as the exp bias
                    nc.vector.reduce_max(nmF[:, j:j + 1], b_new, axis=AX.X,
                                         negate=True)
                for j in range(G):
                    e = sp.tile([128, 16], f32, tag="e")
                    nc.scalar.activation(e, b_cur[j], ACTF.Exp,
                                         bias=nmF[:, j:j + 1], scale=1.0,
                                         accum_out=ZF[:, j:j + 1])
                    t2 = tp.tile([128, 16, O], f32, tag="t2")
                    e_b = e.unsqueeze(2).broadcast_to((128, 16, O))
                    if "gps3" in variant and it == 2:
                        nc.gpsimd.tensor_tensor(t2, u_sb[j], e_b, op=ALU.mult)
                    else:
                        nc.vector.tensor_mul(t2, u_sb[j], e_b)
                    s = sp.tile([128, O], f32, tag="s")
                    nc.vector.reduce_sum(s, t2.transpose([0, 2, 1]), axis=AX.X)
                    s_next.append(s)
                    junk = sp.tile([128, O], f32, tag="junk")
                    nc.scalar.activation(junk, s, ACTF.Square,
                                         accum_out=squF[:, j:j + 1])
                # P5/P7: batched alpha chain
                vsc = alpha_batch(squF, ZF, f"a{it}")
                s_cur = s_next

            # P8: scale + store
            for j, rp in enumerate(grp):
                vt = sp.tile([128, O], f32, tag="vt")
                nc.scalar.mul(vt, s_cur[j], mul=vsc[:, j:j + 1])
                dma_eng.dma_start(out=vout_view[rp], in_=vt)

    nc.compile()
    return nc


def _prep_inputs(x, W):
    """Host-side sharding + layout prep. Returns list of in_maps per core."""
    x = np.ascontiguousarray(x, dtype=np.float32)
    W = np.ascontiguousarray(W, dtype=np.float32)
    in_maps = []
    for k in range(N_CORES):
        r0 = k * R_SHARD
        xs = x[:, r0:r0 + R_SHARD, :]              # [B, 256, I]
        Ws = W[r0:r0 + R_SHARD]                    # [256, C, O, I]

        xw = np.zeros((NPAIR, 32, 672), np.float32)
        # block-diag x stationary: rows (r_hat*16+i), cols (r_hat*64+b)
        xT = xs.transpose(1, 2, 0)                 # [256, I, B]
        xw[:, :16, :64] = xT[0::2]
        xw[:, 16:, 64:128] = xT[1::2]
        # W moving: [:, r_hat*16+i, 128 + c*32+o] = W[r, c, o, i]
        Wt = Ws.transpose(0, 3, 1, 2).reshape(R_SHARD, I, C * O)   # [256, I, 512]
        xw[:, :16, 128:640] = Wt[0::2]
        xw[:, 16:, 128:640] = Wt[1::2]
        wbar = Wt.reshape(R_SHARD, I, C, O).mean(axis=2)           # [256, I, O]
        xw[:, :16, 640:] = wbar[0::2]
        xw[:, 16:, 640:] = wbar[1::2]

        in_maps.append({"xw": xw})
    return in_maps


def kernel(x, W, _trace=False):
    from concourse import bass_utils

    if "nc" not in _cache:
        _cache["nc"] = _build_program()
    nc = _cache["nc"]

    in_maps = _prep_inputs(x, W)
    res = bass_utils.run_bass_kernel_spmd(
        nc, in_maps, core_ids=list(range(N_CORES)), trace=_trace)
    _cache["last_result"] = res

    out = np.empty((B, R, O), np.float32)
    for k in range(N_CORES):
        out[:, k * R_SHARD:(k + 1) * R_SHARD, :] = res.results[k]["vout"]
    return out



# revision 7
# speedup vs baseline: 10.0317x; 10.0317x over previous
"""CapsuleLayer dynamic-routing kernel for 8 Trainium2 NeuronCores.

Problem (hardcoded shapes):
  x: [B=64, R=2048, I=16] f32, W: [R=2048, C=16, O=32, I=16] f32
  u_hat[b,r,c,o] = sum_i W[r,c,o,i] * x[b,r,i]
  3 dynamic-routing iterations (softmax over c, squash over o) -> v [B, R, O]

Strategy (v2):
  - Shard R across 8 cores (256 r's each), chunk = 2 r's x 64 b = 128 rows.
  - PE computes u_hat [128, (c,o)=512] per chunk (block-diag x stationary)
    plus s0 = mean_c u_hat via 32 mean-W moving columns (as v1).
  - Routing on DVE via a CUSTOM segmented-scan op (SEGSUM_TT_ANT):
    out = per-page cumsum of in0*in1 -> fused multiply+segmented-reduce in
    ONE 1x pass (segment sums land at page-last columns, read back strided).
    Four passes per chunk over the 2 live iterations:
      Z: zd[c] = sum_o u[c,o]*s0[o]    (pages=c stride 32, inner o stride 1)
      A: s1[o] = sum_c u[c,o]*e1[c]    (pages=o stride 1, inner c stride 32)
      B: w[c]  = sum_o u[c,o]*s1[o]
      C: s2[o] = sum_c u[c,o]*e2[c]
    b1 = vsc0*zd; ssq1 = e1.w; b2 = b1 + vsc1*w; v = vsc2*s2.
  - All small ops batched G=8 chunks wide on group tiles.
  - Optional GPSIMD offload of pass Z (+ k of 8 pass-B chunks) via stock
    tensor_tensor + tensor_reduce writing the same strided slots.
"""

import numpy as np
import sys

sys.path.insert(0, "/opt/trn_rl_repo")

B, R, C, O, I = 64, 2048, 16, 32, 16
N_CORES = 8
R_SHARD = R // N_CORES          # 256
NPAIR = R_SHARD // 2            # 128 chunks per core
G = 8                           # chunks per group

_cache = {}


# --------------------------------------------------------------------------
# Custom DVE op: per-page-reset multiply-scan (segmented dot product).
# --------------------------------------------------------------------------
def _register_segsum_tt():
    from concourse import dve_spec
    from concourse.dve_spec import (
        Spec, Src0, Src1, Zero, AluOp, scan, lower, _has_src1,
    )
    from concourse.dve_uop import DveOpSpec
    from concourse.dve_table_gen import dve_ver_for
    from concourse.dve_ops import DveOp, OPS, _SUB_OPCODE_FOR_NAME, CUSTOM_DVE_SPECS

    name = "SEGSUM_TT_ANT"
    if name in _SUB_OPCODE_FOR_NAME:
        return next(op for op in OPS if op.name == name)

    if not getattr(dve_spec, "_reset_scan_patched", False):
        dve_spec._RESET_SCAN_IDS = set()
        _orig = dve_spec._scan_overrides

        def _patched(scans, node_stage):
            seed, step = _orig(scans, node_stage)
            for sc in scans:
                if id(sc) in dve_spec._RESET_SCAN_IDS:
                    d = node_stage[sc]
                    init = sc.init if sc.init is not None else Zero
                    # STEP state (1 elem at each SUB_DIM_DONE): d <- op(init, expr)
                    step[d] = dve_spec._Stage(sc.op, init, sc.expr)
            return seed, step

        dve_spec._scan_overrides = _patched
        dve_spec._reset_scan_patched = True

    def _ref(in0, in1, c0, c1, c2):
        x = np.asarray(in0, np.float32)
        y = np.broadcast_to(np.asarray(in1, np.float32), x.shape)
        return np.cumsum((x * y).astype(np.float32), axis=-1, dtype=np.float32)

    sc = scan(AluOp.ADD, Src0 * Src1)
    dve_spec._RESET_SCAN_IDS.add(id(sc))
    spec = Spec(body=sc, reference=_ref)
    spec._keepalive = sc  # keep id() alive

    row = max(_SUB_OPCODE_FOR_NAME.values()) + 1
    assert row < 0x20
    shas = {}
    for ver in {dve_ver_for("TRN2")}:
        s = DveOpSpec(name=name, opcode=row, uops=lower(spec, ver=ver),
                      rd1_en=_has_src1(spec))
        shas[ver] = s.sha(ver)
    op = DveOp(name, spec, subdim=True, uops_sha=shas)
    OPS.append(op)
    _SUB_OPCODE_FOR_NAME[name] = row
    CUSTOM_DVE_SPECS[name] = spec
    return op


def _build_program(npair=NPAIR, reps=1, variant="hwdma"):
    """variant flags: 'hwdma' (sync-engine DMA), 'gpsZBk' (GPSIMD runs pass Z
    plus k of 8 pass-B chunks per group), 'gpsZ' (Z only), plain = no gps."""
    from contextlib import ExitStack

    import concourse.bacc as bacc
    import concourse.tile as tile
    from concourse import mybir

    # Keep every ACT func (Copy/Exp/Square/Ln) in ONE table set to avoid
    # ~2.7us table reloads (see v1 docstring).
    if not getattr(bacc, "_act_tables_patched", False):
        _orig_get_tables = bacc.get_activation_tables

        def _patched(arch):
            tabs = dict(_orig_get_tables(arch))
            target = "natural_log_exp_and_others"
            assert target in tabs
            return {
                name: (funcs if name == target else set())
                for name, funcs in tabs.items()
            }

        bacc.get_activation_tables = _patched
        bacc._act_tables_patched = True

    SEGSUM = _register_segsum_tt()

    f32 = mybir.dt.float32
    AX = mybir.AxisListType
    ALU = mybir.AluOpType
    ACTF = mybir.ActivationFunctionType

    nc = bacc.Bacc("TRN2", target_bir_lowering=False, debug=False)

    xw = nc.dram_tensor("xw", [npair, 32, 672], f32, kind="ExternalInput")
    vout = nc.dram_tensor("vout", [B, 2 * npair, O], f32, kind="ExternalOutput")

    xw_ap = xw.ap()
    # [g, two, b, j, o]: group g covers chunks g*G+j; chunk rows are (two, b)
    vout_view = vout.ap().rearrange(
        "b (g j two) o -> g two b j o", two=2, j=G)

    dma_eng = nc.sync if "hwdma" in variant else nc.gpsimd
    gps_Z, gps_B = False, 0
    if "gpsZB" in variant:
        gps_Z, gps_B = True, int(variant.split("gpsZB")[1][:1])
    elif "gpsZ" in variant:
        gps_Z = True

    n_groups_total = (npair * reps) // G
    assert (npair * reps) % G == 0

    with tile.TileContext(nc) as tc, ExitStack() as ctx:
        xp = ctx.enter_context(tc.tile_pool(name="xp", bufs=2))      # xw loads
        psA = ctx.enter_context(tc.tile_pool(name="psA", bufs=6, space="PSUM"))
        psB = ctx.enter_context(tc.tile_pool(name="psB", bufs=2, space="PSUM"))
        ug = ctx.enter_context(tc.tile_pool(name="ug", bufs=2))      # u mega
        so = ctx.enter_context(tc.tile_pool(name="so", bufs=1))      # scan outs
        gt = ctx.enter_context(tc.tile_pool(name="gt", bufs=2))      # gps tmp
        sm = ctx.enter_context(tc.tile_pool(name="sm", bufs=2))      # small state

        def alpha_batch(squF, ZF, tagp):
            """vsc = alpha*zi batched over group: alpha = sqrt(sig2)/(1+sig2),
            sig2 = squF*zi^2 (zi=1/Z; 1 if ZF None). sqrt via exp(0.5*ln(x))
            keeps every ACT func in one table set."""
            if ZF is not None:
                zi = sm.tile([128, G], f32, tag=tagp + "zi")
                nc.vector.reciprocal(zi, ZF)
                zi2 = sm.tile([128, G], f32, tag=tagp + "zi2")
                nc.vector.tensor_mul(zi2, zi, zi)
                sig2 = sm.tile([128, G], f32, tag=tagp + "sig2")
                nc.vector.tensor_mul(sig2, squF, zi2)
            else:
                sig2 = squF
            a1 = sm.tile([128, G], f32, tag=tagp + "a1")
            nc.vector.tensor_scalar_add(a1, sig2, 1.0)
            ra = sm.tile([128, G], f32, tag=tagp + "ra")
            nc.vector.reciprocal(ra, a1)
            lt = sm.tile([128, G], f32, tag=tagp + "lt")
            nc.scalar.activation(lt, sig2, ACTF.Ln)
            rt = sm.tile([128, G], f32, tag=tagp + "rt")
            nc.scalar.activation(rt, lt, ACTF.Exp, scale=0.5)
            al = sm.tile([128, G], f32, tag=tagp + "al")
            nc.vector.tensor_mul(al, rt, ra)
            if ZF is not None:
                az = sm.tile([128, G], f32, tag=tagp + "az")
                nc.vector.tensor_mul(az, al, zi)
                return az
            return al

        for g in range(n_groups_total):
            gg = g % (npair // G)

            # P0: one DMA for the whole group's xw
            xwt = xp.tile([32, G, 672], f32)
            dma_eng.dma_start(
                out=xwt,
                in_=xw_ap[gg * G:(gg + 1) * G].rearrange("j p k -> p j k"))

            # P1: matmuls (u_hat per chunk + s0 slices into one PSUM tile)
            u_pss = []
            s0_ps = psB.tile([128, G * O], f32, tag="s0ps")
            for j in range(G):
                u_ps = psA.tile([128, 512], f32)
                nc.tensor.matmul(u_ps, lhsT=xwt[:, j, :128],
                                 rhs=xwt[:, j, 128:640], start=True, stop=True)
                nc.tensor.matmul(s0_ps[:, j * O:(j + 1) * O],
                                 lhsT=xwt[:, j, :128], rhs=xwt[:, j, 640:],
                                 start=True, stop=True)
                u_pss.append(u_ps)

            # P2: evacuate to SBUF
            uG = ug.tile([128, G * 512], f32)
            for j, u_ps in enumerate(u_pss):
                nc.scalar.copy(uG[:, j * 512:(j + 1) * 512], u_ps)
            s0G = sm.tile([128, G * O], f32, tag="s0g")
            nc.scalar.copy(s0G, s0_ps)
            s0v = s0G.rearrange("p (j o) -> p j o", o=O)

            # P3: ssq0 -> vsc0
            sq0t = sm.tile([128, G * O], f32, tag="sq0t")
            nc.scalar.activation(sq0t, s0G, ACTF.Square)
            ssq0 = sm.tile([128, G], f32, tag="ssq0")
            nc.vector.reduce_sum(
                ssq0, sq0t.rearrange("p (j o) -> p j o", o=O), axis=AX.X)
            vsc0 = alpha_batch(ssq0, None, "a0")

            def pass_co(in1_of, out_tag, gps_k):
                """contract over o: pages=c stride 32, inner o stride 1."""
                outG = so.tile([128, G * 512], f32, tag=out_tag)
                view = outG.rearrange("p (j c o) -> p j c o", c=C, o=O)
                for j in range(G):
                    in0 = uG[:, j * 512:(j + 1) * 512].rearrange(
                        "p (c o) -> p c o", o=O)
                    in1 = in1_of(j)
                    if j < gps_k:
                        # gpsimd takes the mult; free-dim reduce is DVE-only
                        t = gt.tile([128, C, O], f32, tag="gt_t")
                        nc.gpsimd.tensor_tensor(t, in0, in1, op=ALU.mult)
                        nc.vector.reduce_sum(view[:, j, :, O - 1], t, axis=AX.X)
                    else:
                        nc.vector._custom_dve(
                            SEGSUM, out=view[:, j], in0=in0, in1=in1)
                return view[:, :, :, O - 1]        # [128, G, C] strided

            def pass_cc(eG, out_tag):
                """contract over c: pages=o stride 1, inner c stride 32."""
                outG = so.tile([128, G * 512], f32, tag=out_tag)
                view = outG.rearrange("p (j o c) -> p j o c", o=O, c=C)
                ev = eG.rearrange("p (j c) -> p j c", c=C)
                for j in range(G):
                    in0 = uG[:, j * 512:(j + 1) * 512].rearrange(
                        "p (c o) -> p o c", o=O)
                    in1 = ev[:, j].unsqueeze(1).broadcast_to((128, O, C))
                    nc.vector._custom_dve(
                        SEGSUM, out=view[:, j], in0=in0, in1=in1)
                return view[:, :, :, C - 1]        # [128, G, O] strided

            def softmax(bG, tag):
                bv = bG.rearrange("p (j c) -> p j c", c=C)
                nm = sm.tile([128, G], f32, tag=tag + "nm")
                nc.vector.reduce_max(nm, bv, axis=AX.X, negate=True)
                bs = sm.tile([128, G * C], f32, tag=tag + "bs")
                nc.vector.tensor_add(
                    bs.rearrange("p (j c) -> p j c", c=C), bv,
                    nm.unsqueeze(2).broadcast_to((128, G, C)))
                eG = sm.tile([128, G * C], f32, tag=tag + "e")
                nc.scalar.activation(eG, bs, ACTF.Exp)
                ZG = sm.tile([128, G], f32, tag=tag + "Z")
                nc.vector.reduce_sum(
                    ZG, eG.rearrange("p (j c) -> p j c", c=C), axis=AX.X)
                return eG, ZG

            # P4: pass Z -> b1 = vsc0 * zd
            zd = pass_co(
                lambda j: s0v[:, j].unsqueeze(1).broadcast_to((128, C, O)),
                "zout", G if gps_Z else 0)
            bG1 = sm.tile([128, G * C], f32, tag="b1")
            nc.vector.tensor_mul(
                bG1.rearrange("p (j c) -> p j c", c=C), zd,
                vsc0.unsqueeze(2).broadcast_to((128, G, C)))

            eG1, Z1 = softmax(bG1, "s1")

            # P5: pass A -> s1 (strided [128, G, O])
            s1 = pass_cc(eG1, "aout")

            # P6: pass B -> w; ssq1 = e1.w; b2 = b1 + vsc1*w
            w = pass_co(
                lambda j: s1[:, j].unsqueeze(1).broadcast_to((128, C, O)),
                "bout", gps_B)
            ew = sm.tile([128, G * C], f32, tag="ew")
            ewv = ew.rearrange("p (j c) -> p j c", c=C)
            nc.vector.tensor_mul(
                ewv, eG1.rearrange("p (j c) -> p j c", c=C), w)
            ssq1 = sm.tile([128, G], f32, tag="ssq1")
            nc.vector.reduce_sum(ssq1, ewv, axis=AX.X)
            vsc1 = alpha_batch(ssq1, Z1, "a1")
            wv = sm.tile([128, G * C], f32, tag="wv")
            nc.vector.tensor_mul(
                wv.rearrange("p (j c) -> p j c", c=C), w,
                vsc1.unsqueeze(2).broadcast_to((128, G, C)))
            bG2 = sm.tile([128, G * C], f32, tag="b2")
            nc.vector.tensor_add(bG2, bG1, wv)

            eG2, Z2 = softmax(bG2, "s2")

            # P7: pass C -> s2
            s2 = pass_cc(eG2, "cout")

            # P8: ssq2 -> vsc2; v = vsc2*s2; store
            sqc = sm.tile([128, G * O], f32, tag="sqc")
            nc.scalar.activation(
                sqc.rearrange("p (j o) -> p j o", o=O), s2, ACTF.Square)
            ssq2 = sm.tile([128, G], f32, tag="ssq2")
            nc.vector.reduce_sum(
                ssq2, sqc.rearrange("p (j o) -> p j o", o=O), axis=AX.X)
            vsc2 = alpha_batch(ssq2, Z2, "a2")
            vtG = sm.tile([128, G * O], f32, tag="vt")
            nc.vector.tensor_mul(
                vtG.rearrange("p (j o) -> p j o", o=O), s2,
                vsc2.unsqueeze(2).broadcast_to((128, G, O)))
            vtv = vtG.rearrange("p (j o) -> p j o", o=O)
            dma_eng.dma_start(out=vout_view[gg, 0], in_=vtv[:64])
            dma_eng.dma_start(out=vout_view[gg, 1], in_=vtv[64:])

    nc.compile()
    return nc


def _prep_inputs(x, W):
    """Host-side sharding + layout prep. Returns list of in_maps per core."""
    x = np.ascontiguousarray(x, dtype=np.float32)
    W = np.ascontiguousarray(W, dtype=np.float32)
    in_maps = []
    for k in range(N_CORES):
        r0 = k * R_SHARD
        xs = x[:, r0:r0 + R_SHARD, :]              # [B, 256, I]
        Ws = W[r0:r0 + R_SHARD]                    # [256, C, O, I]

        xw = np.zeros((NPAIR, 32, 672), np.float32)
        xT = xs.transpose(1, 2, 0)                 # [256, I, B]
        xw[:, :16, :64] = xT[0::2]
        xw[:, 16:, 64:128] = xT[1::2]
        Wt = Ws.transpose(0, 3, 1, 2).reshape(R_SHARD, I, C * O)   # [256, I, 512]
        xw[:, :16, 128:640] = Wt[0::2]
        xw[:, 16:, 128:640] = Wt[1::2]
        wbar = Wt.reshape(R_SHARD, I, C, O).mean(axis=2)           # [256, I, O]
        xw[:, :16, 640:] = wbar[0::2]
        xw[:, 16:, 640:] = wbar[1::2]

        in_maps.append({"xw": xw})
    return in_maps


def kernel(x, W, _trace=False):
    from concourse import bass_utils

    if "nc" not in _cache:
        _cache["nc"] = _build_program()
    nc = _cache["nc"]

    in_maps = _prep_inputs(x, W)
    res = bass_utils.run_bass_kernel_spmd(
        nc, in_maps, core_ids=list(range(N_CORES)), trace=_trace)
    _cache["last_result"] = res

    out = np.empty((B, R, O), np.float32)
    for k in range(N_CORES):
        out[:, k * R_SHARD:(k + 1) * R_SHARD, :] = res.results[k]["vout"]
    return out
